# revision 1
# baseline (speedup 1.0000x reference)
"""MixGARCH Trainium2 kernel.

Reference semantics: scan over t of
    v_t = relu(bias + Wx @ o_t^2 + Wh * v_{t-1}) + 1e-6,  hist[t] = v_t
with bias, Wx, Wh, o^2, v0 all >= 0, so relu is an identity and this is a
LINEAR first-order recurrence:
    v_t = Wh * v_{t-1} + c_t,   c_t = (bias + 1e-6) + Wx @ o_t^2

Strategy (8 cores, full I/O):
 - Each core owns 65536 timesteps, split into 2 halves of 32768. Each half is
   an independent scan lane group (64 components), giving 128 SBUF partitions
   of independent recurrences per core.
 - Cross-boundary state is handled with a 1024-step warmup (Wh < 0.9, so the
   influence of the unknown incoming state decays below fp32 resolution in
   <600 steps; 0.9^1024 ~ 1e-47 == 0.0f). Core 0 half A starts from the exact
   v0 instead (no warmup).
 - On device: PE matmuls compute Wx @ o^2 (zero-padded 32-row weight variants,
   so every matmul is 32-partition aligned), ACT squares the input, copies
   PSUM->SBUF adding (bias + 1e-6) per partition, and DVE tensor_tensor_scan
   runs the recurrence 128 lanes at a time, chained across 512-wide tiles.
 - Host packs the input into the exact SBUF layout (128 = 16 chunks x 8
   channels) and de-interleaves the (128, T'') output back to (T, 64).
"""

import os
import numpy as np

T = 524288
K = 64
NJ = 8
NCORES = 8
W = 1024              # warmup steps per half
HALF = 32768          # real steps per half
TT = W + HALF         # 33792 = per-half scan length
NCH = 8               # chunks per half
CHUNK = TT // NCH     # 4224 elements per partition
F = 512               # scan tile width
NTILES = TT // F      # 66
STAGE = 8 * F         # 4096-wide output staging
MM_DT = os.environ.get("MIXGARCH_MM_DTYPE", "float32")

_CACHE = {}


def _build_nc():
    import concourse.bacc as bacc
    import concourse.mybir as mybir
    import concourse.tile as tile

    mm_dt = getattr(mybir.dt, MM_DT)
    f32 = mybir.dt.float32
    PSUM_BUFS = 6
    CSB_BUFS = 8

    nc = bacc.Bacc(None, target_bir_lowering=False)
    xin = nc.dram_tensor("xin", [128, CHUNK], f32, kind="ExternalInput")
    wt = nc.dram_tensor("wt", [128, 256], f32, kind="ExternalInput")
    biast = nc.dram_tensor("biast", [128, 1], f32, kind="ExternalInput")
    wscan = nc.dram_tensor("wscan", [128, F], f32, kind="ExternalInput")
    vinit = nc.dram_tensor("vinit", [128, 1], f32, kind="ExternalInput")
    vout = nc.dram_tensor("vout", [128, TT], f32, kind="ExternalOutput")

    with tile.TileContext(nc) as tc:
        with (
            tc.tile_pool(name="const", bufs=1) as cpool,
            tc.tile_pool(name="xbuf", bufs=1) as xpool,
            tc.tile_pool(name="cbuf", bufs=1) as cbuf,
            tc.tile_pool(name="stage", bufs=2) as stpool,
            tc.tile_pool(name="psum", bufs=1, space="PSUM") as ps,
        ):
            wt_sb = cpool.tile([128, 256], f32)
            nc.sync.dma_start(wt_sb[:], wt[:])
            bias_sb = cpool.tile([128, 1], f32)
            nc.sync.dma_start(bias_sb[:], biast[:])
            ws_sb = cpool.tile([128, F], f32)
            nc.sync.dma_start(ws_sb[:], wscan[:])
            vi_sb = cpool.tile([128, 1], f32)
            nc.sync.dma_start(vi_sb[:], vinit[:])

            x_sb = xpool.tile([128, CHUNK], f32)
            x2_sb = xpool.tile([128, CHUNK], mm_dt)
            if MM_DT != "float32":
                wt_mm = cpool.tile([128, 256], mm_dt)
                nc.scalar.activation(
                    wt_mm[:], wt_sb[:], mybir.ActivationFunctionType.Copy
                )
            else:
                wt_mm = wt_sb
            NLOAD = 4
            lw = CHUNK // NLOAD  # 1056
            for q in range(NLOAD):
                sl = slice(q * lw, (q + 1) * lw)
                nc.sync.dma_start(x_sb[:, sl], xin[:, sl])
                nc.scalar.activation(
                    x2_sb[:, sl], x_sb[:, sl], mybir.ActivationFunctionType.Square
                )

            prev_stage = None
            stage_t = None
            for i in range(NTILES):
                slot = i % 8
                if slot == 0:
                    prev_stage = stage_t
                    nst = STAGE if (NTILES - i) >= 8 else (NTILES - i) * F
                    stage_t = stpool.tile([128, nst], f32, tag="stage")

                # Per-slot tags pin PSUM reuse to exactly i - PSUM_BUFS.
                c_ps = ps.tile([128, F], f32, tag=f"cps{i % PSUM_BUFS}")
                for h in range(2):
                    done = 0
                    while done < F:
                        pos = i * F + done
                        c = pos // CHUNK
                        off = pos % CHUNK
                        n = min(F - done, CHUNK - off, 512)
                        g = h * NCH + c
                        b, r = g // 4, g % 4
                        lhsT = wt_mm[32 * b:32 * b + 32, 64 * r:64 * r + 64]
                        rhs = x2_sb[32 * b:32 * b + 32, off:off + n]
                        nc.tensor.matmul(
                            c_ps[64 * h:64 * h + 64, done:done + n],
                            lhsT,
                            rhs,
                            start=True,
                            stop=True,
                            tile_position=(32 * b, 64 * h),
                        )
                        done += n

                c_sb = cbuf.tile([128, F], f32, tag=f"csb{i % CSB_BUFS}")
                nc.scalar.activation(
                    c_sb[:], c_ps[:], mybir.ActivationFunctionType.Identity,
                    bias=bias_sb[:, 0:1],
                )

                initial = (
                    vi_sb[:, 0:1]
                    if i == 0
                    else (
                        stage_t[:, slot * F - 1:slot * F]
                        if slot > 0
                        else prev_stage[:, prev_stage.shape[1] - 1:prev_stage.shape[1]]
                    )
                )
                nc.vector.tensor_tensor_scan(
                    stage_t[:, slot * F:(slot + 1) * F],
                    ws_sb[:],
                    c_sb[:],
                    initial,
                    mybir.AluOpType.mult,
                    mybir.AluOpType.add,
                )

                if slot == 7 or i == NTILES - 1:
                    base = (i - slot) * F
                    nc.sync.dma_start(
                        vout[:, base:base + stage_t.shape[1]], stage_t[:]
                    )

    nc.compile()
    return nc


def _host_prep(series, vars0, bias, Wx, Wh):
    series = np.asarray(series, dtype=np.float32)
    vars0 = np.asarray(vars0, dtype=np.float32)
    bias = np.asarray(bias, dtype=np.float32)
    Wx = np.asarray(Wx, dtype=np.float32)
    Wh = np.asarray(Wh, dtype=np.float32)

    in_maps = []
    wt = np.zeros((128, 256), dtype=np.float32)
    for q in range(4):
        for r in range(4):
            for j in range(NJ):
                wt[32 * q + 8 * r + j, 64 * r:64 * r + 64] = Wx[:, j]
    biasv = np.zeros((128, 1), dtype=np.float32)
    biasv[0:64, 0] = bias + 1e-6
    biasv[64:128, 0] = bias + 1e-6
    wscan = np.zeros((128, F), dtype=np.float32)
    wscan[0:64, :] = Wh[:, None]
    wscan[64:128, :] = Wh[:, None]

    for i in range(NCORES):
        xin = np.empty((128, CHUNK), dtype=np.float32)
        for h in range(2):
            start = i * 65536 + h * HALF
            if i == 0 and h == 0:
                rows = series[0:TT]
            else:
                rows = series[start - W:start + HALF]
            for c in range(NCH):
                g = h * NCH + c
                b, r = g // 4, g % 4
                xin[32 * b + 8 * r:32 * b + 8 * r + 8, :] = (
                    rows[c * CHUNK:(c + 1) * CHUNK, :].T
                )
        vinit = np.zeros((128, 1), dtype=np.float32)
        if i == 0:
            vinit[0:64, 0] = vars0
        in_maps.append(
            {"xin": xin, "wt": wt, "biast": biasv, "wscan": wscan, "vinit": vinit}
        )
    return in_maps


def _assemble(results):
    hist = np.empty((T, K), dtype=np.float32)
    for i in range(NCORES):
        vout = results[i]["vout"]
        for h in range(2):
            start = i * 65536 + h * HALF
            q0 = 0 if (i == 0 and h == 0) else W
            hist[start:start + HALF, :] = vout[64 * h:64 * h + 64,
                                               q0:q0 + HALF].T
    return hist


def run(inputs, trace=False, **kw):
    from concourse.bass_utils import run_bass_kernel_spmd

    if "nc" not in _CACHE:
        _CACHE["nc"] = _build_nc()
    nc = _CACHE["nc"]
    in_maps = _host_prep(
        inputs["series"], inputs["vars0"], inputs["bias"],
        inputs["Wx"], inputs["Wh"],
    )
    res = run_bass_kernel_spmd(
        nc, in_maps, core_ids=list(range(NCORES)), trace=trace, **kw
    )
    return _assemble(res.results), res


def kernel(series, vars0, bias, Wx, Wh):
    out, _ = run(
        {"series": series, "vars0": vars0, "bias": bias, "Wx": Wx, "Wh": Wh}
    )
    return out



# revision 4
# speedup vs baseline: 1.8276x; 1.8276x over previous
"""MixGARCH Trainium2 kernel — unroll-by-4 linear-scan architecture.

Reference semantics: scan over t of
    v_t = relu(bias + Wx @ o_t^2 + Wh * v_{t-1}) + 1e-6,  hist[t] = v_t
with bias, Wx, Wh, o^2, v0 all >= 0, so relu is an identity and this is a
LINEAR first-order recurrence:
    v_t = Wh * v_{t-1} + c_t,   c_t = (bias + 1e-6) + Wx @ o_t^2

Unrolled by U=4, the recurrence at stride 4 is
    V_i = v_{t0+4i} = w^4 * V_{i-1} + d_i,
    d_i = sum_{m=0..3} w^m c_{t0+4i-m}
and the intermediate phases j=1..3 are
    v_{t0+4i+j} = sum_{m=0..j-1} w^m c_{t0+4i+j-m} + w^j V_i.

Mapping to engines (per core, halves stacked on partitions):
 - PE computes d (one matmul: 68 input rows = 2 halves x (8 ch x 4 lags +
   ones row for bias + init row for exact v0 injection)), and the phase
   reconstruction (partial-sum matmul over lagged inputs + diagonal w^j
   matmul against the scan output V, accumulated in one PSUM tile).
 - ACT copies d PSUM->SBUF (fp16); DVE runs tensor_tensor_scan over TU
   columns only (T/4); phase tiles are copied PSUM->SBUF fp16 by ACT/DVE.
 - All DMA traffic is fp16.

Each half has W=2048 warmup steps (w<0.9 ⇒ (w^4)^512 == 0.0f); core 0
half 0 instead starts exactly from vars0 via the init row.
"""

import numpy as np

T = 524288
K = 64
NJ = 8
NCORES = 8
HALF = 32768
W = 2048              # warmup steps (real time) per half
U = 4                 # unroll factor
TU = (HALF + W) // U  # 8704 scan columns per half-timeline
XCOLS = TU + 1        # x2ph columns (partials read one column ahead)
F = 512               # window (PSUM tile) width
NW = TU // F          # 17
GRP = 4               # windows per output DMA group

_CACHE = {}


# ---------------------------------------------------------------------------
# Host-side packing
# ---------------------------------------------------------------------------

def _weights(bias, Wx, Wh, vars0):
    """Build the [128, 768] fp16 stationary-weight pack (shared by cores).

    Column blocks of 128: LTd | LTpA | LTpB | LTpC | LTq12 | LTq3.
    Row layout (contraction partitions): half h at base h*34:
      rows +m*8+n : o^2 channel n at lag m   (m=0..3)
      row  +32    : ones (bias)
      row  +33    : init (exact-v0 injection; used by core0 h0 col 0 only)
    """
    w = Wh.astype(np.float64)
    b = (bias.astype(np.float64) + 1e-6)
    Wxd = Wx.astype(np.float64)
    v0 = vars0.astype(np.float64)
    wp = [w**m for m in range(5)]  # wp[m] = w^m

    wts = np.zeros((128, 768), dtype=np.float64)
    for h in range(2):
        hb = h * 34
        oc = h * 64
        # --- LTd (cols 0:128): d_i = sum_m w^m c_{4i-m}
        for m in range(4):
            for n in range(NJ):
                wts[hb + m * 8 + n, 0 + oc:0 + oc + K] = wp[m] * Wxd[:, n]
        wts[hb + 32, 0 + oc:0 + oc + K] = b * (wp[0] + wp[1] + wp[2] + wp[3])
        wts[hb + 33, 0 + oc:0 + oc + K] = w * v0 - b * (wp[1] + wp[2] + wp[3])
        # --- LTpA (cols 128:256) for h==0 / LTpB (cols 256:384) for h==1:
        # phase partials j=1 (out rows 0:64) and j=2 (out rows 64:128),
        # reading x2ph column i+1 (times tau+4-m').
        pc = 128 + 128 * h
        for j, jo in ((1, 0), (2, 64)):
            for mp in range(4 - j, 4):      # m' = 4-j .. 3
                coef = wp[mp - (4 - j)]     # w^(m'-(4-j))
                for n in range(NJ):
                    wts[hb + mp * 8 + n, pc + jo:pc + jo + K] = coef * Wxd[:, n]
            wts[hb + 32, pc + jo:pc + jo + K] = b * sum(wp[m] for m in range(j))
        # --- LTpC (cols 384:512): phase j=3, h0 -> out rows 0:64,
        # h1 -> out rows 64:128.
        j = 3
        jo = h * 64
        for mp in range(4 - j, 4):
            coef = wp[mp - (4 - j)]
            for n in range(NJ):
                wts[hb + mp * 8 + n, 384 + jo:384 + jo + K] = coef * Wxd[:, n]
        wts[hb + 32, 384 + jo:384 + jo + K] = b * sum(wp[m] for m in range(j))
    # --- LTq12 (cols 512:640): diagonal w (out rows 0:64) and w^2
    # (out rows 64:128); contraction rows 0:64 serve tile A (V of h0),
    # rows 64:128 serve tile B (V of h1).
    for r0 in (0, 64):
        for k in range(K):
            wts[r0 + k, 512 + k] = wp[1][k]
            wts[r0 + k, 512 + 64 + k] = wp[2][k]
    # --- LTq3 (cols 640:768): diagonal w^3 for both halves.
    for r0 in (0, 64):
        for k in range(K):
            wts[r0 + k, 640 + r0 + k] = wp[3][k]
    return wts.astype(np.float16)


def _host_prep(series, vars0, bias, Wx, Wh):
    series = np.asarray(series, dtype=np.float32)
    vars0 = np.asarray(vars0, dtype=np.float32)
    bias = np.asarray(bias, dtype=np.float32)
    Wx = np.asarray(Wx, dtype=np.float32)
    Wh = np.asarray(Wh, dtype=np.float32)

    sq = np.zeros((4 + T + 8, NJ), dtype=np.float32)
    sq[4:4 + T] = series * series
    sqh = sq.astype(np.float16)

    wts = _weights(bias, Wx, Wh, vars0)
    w4 = (Wh.astype(np.float64) ** 4).astype(np.float16)
    ws = np.zeros((128, F), dtype=np.float16)
    ws[0:64, :] = w4[:, None]
    ws[64:128, :] = w4[:, None]

    in_maps = []
    for c in range(NCORES):
        x2 = np.zeros((68, XCOLS), dtype=np.float16)
        for h in range(2):
            hb = h * 34
            special = (c == 0 and h == 0)
            t0 = 0 if special else c * 65536 + h * HALF - W
            for m in range(4):
                s = t0 - m + 4
                for n in range(NJ):
                    x2[hb + m * 8 + n, :] = sqh[s:s + 4 * XCOLS:4, n]
            x2[hb + 32, :] = 1.0
            if special:
                x2[hb + 33, 0] = 1.0
        in_maps.append({"x2ph": x2, "wts": wts, "wscan": ws})
    return in_maps


def _assemble(results):
    hist = np.empty((T, K), dtype=np.float32)
    for c in range(NCORES):
        vv = results[c]["vout_v"].astype(np.float32)
        va = results[c]["vout_a"].astype(np.float32)
        vb = results[c]["vout_b"].astype(np.float32)
        vc = results[c]["vout_c"].astype(np.float32)
        for h in range(2):
            special = (c == 0 and h == 0)
            hs = c * 65536 + h * HALF
            i0 = 0 if special else W // U
            sl = slice(i0, i0 + HALF // U)
            ph12 = va if h == 0 else vb
            hist[hs + 0:hs + HALF:4, :] = vv[h * 64:h * 64 + 64, sl].T
            hist[hs + 1:hs + HALF:4, :] = ph12[0:64, sl].T
            hist[hs + 2:hs + HALF:4, :] = ph12[64:128, sl].T
            hist[hs + 3:hs + HALF:4, :] = vc[h * 64:h * 64 + 64, sl].T
    return hist


# ---------------------------------------------------------------------------
# Numpy emulation of the device dataflow (validation aid)
# ---------------------------------------------------------------------------

def _emulate(inputs):
    """Emulate the device kernel in numpy (fp32 accumulation, fp16
    storage) using the exact packed tensors; returns assembled hist."""
    in_maps = _host_prep(
        inputs["series"], inputs["vars0"], inputs["bias"],
        inputs["Wx"], inputs["Wh"],
    )
    results = []
    for c in range(NCORES):
        x2 = in_maps[c]["x2ph"].astype(np.float32)
        wts = in_maps[c]["wts"].astype(np.float32)
        ws = in_maps[c]["wscan"].astype(np.float32)
        d = (wts[0:68, 0:128].T @ x2).astype(np.float16).astype(np.float32)
        # On HW the scan state stays fp32 across steps; only out is fp16.
        state = np.zeros(128, dtype=np.float32)
        svf = np.empty((128, TU), dtype=np.float32)
        for i in range(TU):
            state = ws[:, 0] * state + d[:, i]
            svf[:, i] = state
        sv = svf.astype(np.float16)
        svf32 = sv.astype(np.float32)
        va = wts[0:68, 128:256].T @ x2[:, 1:] + wts[0:64, 512:640].T @ svf32[0:64]
        vb = wts[0:68, 256:384].T @ x2[:, 1:] + wts[64:128, 512:640].T @ svf32[64:128]
        vc = wts[0:68, 384:512].T @ x2[:, 1:] + wts[0:128, 640:768].T @ svf32
        results.append({
            "vout_v": sv,
            "vout_a": va.astype(np.float16),
            "vout_b": vb.astype(np.float16),
            "vout_c": vc.astype(np.float16),
        })
    return _assemble(results)


# ---------------------------------------------------------------------------
# Bass kernel
# ---------------------------------------------------------------------------

def _build_nc():
    import concourse.bacc as bacc
    import concourse.mybir as mybir
    import concourse.tile as tile

    f32 = mybir.dt.float32
    f16 = mybir.dt.float16

    nc = bacc.Bacc(None, target_bir_lowering=False)
    x2d = nc.dram_tensor("x2ph", [68, XCOLS], f16, kind="ExternalInput")
    wtd = nc.dram_tensor("wts", [128, 768], f16, kind="ExternalInput")
    wsd = nc.dram_tensor("wscan", [128, F], f16, kind="ExternalInput")
    vv = nc.dram_tensor("vout_v", [128, TU], f16, kind="ExternalOutput")
    va = nc.dram_tensor("vout_a", [128, TU], f16, kind="ExternalOutput")
    vb = nc.dram_tensor("vout_b", [128, TU], f16, kind="ExternalOutput")
    vc = nc.dram_tensor("vout_c", [128, TU], f16, kind="ExternalOutput")
    vouts = {"a": va, "b": vb, "c": vc}

    AF = mybir.ActivationFunctionType
    ALU = mybir.AluOpType

    with tile.TileContext(nc) as tc:
        with (
            tc.tile_pool(name="const", bufs=1) as cpool,
            tc.tile_pool(name="big", bufs=1) as bpool,
            tc.tile_pool(name="stage", bufs=2) as stpool,
            tc.tile_pool(name="psum", bufs=1, space="PSUM") as ps,
        ):
            wt_sb = cpool.tile([128, 768], f16)
            nc.sync.dma_start(wt_sb[:], wtd[:])
            ws_sb = cpool.tile([128, F], f16)
            nc.sync.dma_start(ws_sb[:], wsd[:])

            x2_sb = bpool.tile([68, XCOLS], f16)
            XCH = [(0, 2177), (2177, 2176), (4353, 2176), (6529, 2176)]
            for s, n in XCH:
                nc.sync.dma_start(x2_sb[:, s:s + n], x2d[:, s:s + n])

            d_sb = bpool.tile([128, TU], f16)
            sv_sb = bpool.tile([128, TU], f16)

            def emit_d_scan(w):
                win = slice(w * F, (w + 1) * F)
                ps_d = ps.tile([128, F], f32, tag=f"d{w % 2}")
                nc.tensor.matmul(
                    ps_d[:], wt_sb[0:68, 0:128], x2_sb[0:68, win],
                    start=True, stop=True,
                )
                nc.scalar.activation(d_sb[:, win], ps_d[:], AF.Identity)
                initial = 0.0 if w == 0 else sv_sb[:, w * F - 1:w * F]
                nc.vector.tensor_tensor_scan(
                    sv_sb[:, win], ws_sb[:], d_sb[:, win], initial,
                    ALU.mult, ALU.add,
                )

            stg = {}
            emit_d_scan(0)
            for w in range(NW):
                if w + 1 < NW:
                    emit_d_scan(w + 1)
                win = slice(w * F, (w + 1) * F)
                win1 = slice(w * F + 1, (w + 1) * F + 1)
                g, gi = w // GRP, w % GRP
                if gi == 0:
                    gw = min(GRP, NW - w)
                    for x in ("a", "b", "c"):
                        stg_t = stpool.tile([128, gw * F], f16, tag=f"g{x}")
                        stg[x] = stg_t
                ssl = slice(gi * F, (gi + 1) * F)

                ps_a = ps.tile([128, F], f32, tag=f"a{w % 2}")
                nc.tensor.matmul(ps_a[:], wt_sb[0:68, 128:256],
                                 x2_sb[0:68, win1], start=True, stop=False)
                nc.tensor.matmul(ps_a[:], wt_sb[0:64, 512:640],
                                 sv_sb[0:64, win], start=False, stop=True)
                nc.scalar.activation(stg["a"][:, ssl], ps_a[:], AF.Identity)

                ps_b = ps.tile([128, F], f32, tag=f"b{w % 2}")
                nc.tensor.matmul(ps_b[:], wt_sb[0:68, 256:384],
                                 x2_sb[0:68, win1], start=True, stop=False)
                nc.tensor.matmul(ps_b[:], wt_sb[64:128, 512:640],
                                 sv_sb[64:128, win], start=False, stop=True)
                nc.vector.tensor_copy(stg["b"][:, ssl], ps_b[:])

                ps_c = ps.tile([128, F], f32, tag=f"c{w % 2}")
                nc.tensor.matmul(ps_c[:], wt_sb[0:68, 384:512],
                                 x2_sb[0:68, win1], start=True, stop=False)
                nc.tensor.matmul(ps_c[:], wt_sb[0:128, 640:768],
                                 sv_sb[:, win], start=False, stop=True)
                if w % 2 == 0:
                    nc.scalar.activation(stg["c"][:, ssl], ps_c[:], AF.Identity)
                else:
                    nc.vector.tensor_copy(stg["c"][:, ssl], ps_c[:])

                if gi == GRP - 1 or w == NW - 1:
                    base = g * GRP * F
                    for x in ("a", "b", "c"):
                        nc.sync.dma_start(
                            vouts[x][:, base:base + stg[x].shape[1]], stg[x][:]
                        )

                # V output DMA in 4 chunks as the scan completes slices.
                if w in (3, 7, 11, NW - 1):
                    VCH = {3: (0, 2048), 7: (2048, 2048),
                           11: (4096, 2048), NW - 1: (6144, TU - 6144)}
                    s, n = VCH[w]
                    nc.sync.dma_start(vv[:, s:s + n], sv_sb[:, s:s + n])

    nc.compile()
    return nc


def run(inputs, trace=False, **kw):
    from concourse.bass_utils import run_bass_kernel_spmd

    if "nc" not in _CACHE:
        _CACHE["nc"] = _build_nc()
    nc = _CACHE["nc"]
    in_maps = _host_prep(
        inputs["series"], inputs["vars0"], inputs["bias"],
        inputs["Wx"], inputs["Wh"],
    )
    res = run_bass_kernel_spmd(
        nc, in_maps, core_ids=list(range(NCORES)), trace=trace, **kw
    )
    return _assemble(res.results), res


def kernel(series, vars0, bias, Wx, Wh):
    out, _ = run(
        {"series": series, "vars0": vars0, "bias": bias, "Wx": Wx, "Wh": Wh}
    )
    return out
